# revision 1
# baseline (speedup 1.0000x reference)
"""Single-head attention (B=4, L=4096, EMB=312, HID=256) on 8 NeuronCores.

Sharding: data-parallel over batch (4) x key-parallel (2) = 8 cores. Each
core handles ALL 4096 queries against its half of the keys and returns the
UNNORMALIZED partial [sum_k p*v | sum_k p] rows; the host combines the two
halves as (o1+o2)/(s1+s2). Key-sharding (vs query-sharding) halves the
duplicated K/V projection work; only the Q projection is duplicated.

Per-core device algorithm:
  - Host sends transposed, padded inputs split into bf16 (hi, lo) pairs; a
    matmul A@B is computed as A_hi@B_hi + A_lo@B_hi + A_hi@B_lo (the dropped
    lo@lo term is ~2^-18 relative), giving ~fp32-quality products at the
    bf16 PE rate (1 cycle/row).
  - embT carries a ones-row at index EMB and W* carry the bias in that row,
    so projections fold the bias in. Wv has 2 extra columns: ones (gives the
    softmax row-sum through the P@V matmul) and zero padding (even N).
  - Scores are computed transposed: sT[kl, ql] = kT-chunk^T @ qT, so the
    exp() output is directly the stationary operand for the P@V matmul —
    no on-device transposes anywhere.
  - Mask is host-side transposed, scaled by -1e5, cast to bf16; applied
    additively to the score PSUM by the vector engine. exp() on the scalar
    engine. The raw partials (P@V columns + row-sum column) go back to the
    host, which normalizes after combining the key-halves.

Precision ladder (BASS_KERNEL_PRECISION); projections are always bf16x2.
Measured absmax error relative to max|output| and TimelineSim time/core:
  - "fast":     fp32r single-pass QK and PV          ~9.4e-4 of scale
  - "fp16qk":   fp16 single-pass QK, fp32r PV        ~1.6e-3 of scale
  - "balanced": bf16x2 3-term QK, fp32r PV           ~1.6e-4, ~312 us  (default)
  - "exact":    also bf16x2 p/v in PV                ~3.8e-5, ~440 us
"""
import os

import numpy as np
import ml_dtypes

import concourse.bacc as bacc
import concourse.tile as tile
from concourse import mybir, bass2jax
from concourse.bass_utils import run_bass_kernel_spmd

# Debug aid (opt-in): surface real compile errors from the PJRT compile
# hook, which the C++ bridge otherwise swallows.
if os.environ.get("BASS_KERNEL_DEBUG"):
    import functools as _ft
    import traceback as _tb
    _orig_hook = bass2jax.neuronx_cc_hook
    @_ft.wraps(_orig_hook)
    def _dbg_hook(*args, **kwargs):
        try:
            return _orig_hook(*args, **kwargs)
        except BaseException:
            _tb.print_exc()
            raise
    bass2jax.neuronx_cc_hook = _dbg_hook

EMB, HID, B, L = 312, 256, 4, 4096
NCORES = 8
P = 128
KL = L // 2            # key rows per core (key-parallel halves)
EPAD = 384             # emb dim padded to 3 partition chunks; row EMB is the ones-row
HV = HID + 2           # v columns: HID values | ones | zero pad (even N for matmul)
QT = 512               # ql tile width (PSUM bank = 512 fp32)
NKC = KL // P          # 16 kl chunks per core
NQTT = L // QT         # 8 ql tiles per core (all queries)
NKT = KL // QT         # 4 l tiles for the k projection
MASK_SCALE = np.float32(-100000.0)

F32 = mybir.dt.float32
F16 = mybir.dt.float16
F32R = mybir.dt.float32r
BF16 = mybir.dt.bfloat16
BF = ml_dtypes.bfloat16

_CACHE = {}

# (lhs_piece, rhs_piece) index pairs for the 3-term bf16x2 product.
SPLIT3 = ((0, 0), (1, 0), (0, 1))


def _build(precision):
    qk_exact = precision in ("balanced", "exact")
    qk_fp16 = precision == "fp16qk"
    pv_exact = precision == "exact"

    nc = bacc.Bacc(None)

    def dram_pair(name, shape):
        return tuple(
            nc.dram_tensor(f"{name}{s}", shape, BF16, kind="ExternalInput")
            for s in ("_hi", "_lo")
        )

    embT = dram_pair("embT", [EPAD, L])
    embTk = dram_pair("embTk", [EPAD, KL])
    wq = dram_pair("wq", [EPAD, HID])
    wk = dram_pair("wk", [EPAD, HID])
    wv = dram_pair("wv", [EPAD, HV])
    maskT = nc.dram_tensor("maskT", [KL, L], BF16, kind="ExternalInput")
    out = nc.dram_tensor("out", [L, HID + 1], F32, kind="ExternalOutput")

    with tile.TileContext(nc) as tc:
        with (
            tc.tile_pool(name="big", bufs=1) as big,
            tc.tile_pool(name="wp", bufs=1) as wp,
            tc.tile_pool(name="mt", bufs=10) as mtp,
            tc.tile_pool(name="pt", bufs=4) as ptp,
            tc.tile_pool(name="fin", bufs=4) as fin,
            tc.tile_pool(name="ps_st", bufs=4, space="PSUM") as ps_st,
            tc.tile_pool(name="ps_pv", bufs=1, space="PSUM") as ps_pv,
        ):
            # ---- load inputs (as [P, chunk, free] with the chunk index in
            # the free dim; partition line p reads rows {p, 128+p, 256+p}).
            # Large tensors are loaded in column blocks, lowest columns first
            # across all chunks, so the first projection matmuls can start
            # ~2us in instead of waiting for the whole 6 MB transfer.
            def load_pair(pool, name, dram, ncol, blk=None):
                ts = [
                    pool.tile([P, 3, ncol], BF16, name=f"{name}_{s}", tag=f"{name}_{s}")
                    for s in ("hi", "lo")
                ]
                if blk is None:
                    # Small (weight) loads ride the second HWDGE ring (ACT)
                    # so they don't serialize ahead of the first embTk
                    # blocks on the SP ring at startup.
                    for t, d in zip(ts, dram):
                        nc.scalar.dma_start(out=t, in_=d[:, :].rearrange("(c p) n -> p c n", p=P))
                else:
                    # hi and lo interleaved per column block: the 3-term
                    # projection of block b needs both pieces of block b.
                    for b0 in range(0, ncol, blk):
                        for c in range(3):
                            for t, d in zip(ts, dram):
                                nc.sync.dma_start(
                                    out=t[:, c, b0:b0 + blk],
                                    in_=d[c * P:(c + 1) * P, b0:b0 + blk],
                                )
                return tuple(ts)

            wq_t = load_pair(wp, "wq", wq, HID)
            wk_t = load_pair(wp, "wk", wk, HID)
            wv_t = load_pair(wp, "wv", wv, HV)
            # embTk first: the projection phase starts with k/v tiles,
            # which consume the key-half slice.
            embTk_t = load_pair(big, "embTk", embTk, KL, blk=QT)
            embT_t = load_pair(big, "embT", embT, L, blk=QT)

            def mm3(ps, lhs_pair, rhs_pair, lslice, rslice):
                """ps = sum over 3 e-chunks of (lhs @ rhs) in bf16x2 3-term form."""
                n = len(SPLIT3) * 3
                i = 0
                for a, b in SPLIT3:
                    for e in range(3):
                        nc.tensor.matmul(
                            ps,
                            lhsT=lhs_pair[a][(slice(None), e) + lslice],
                            rhs=rhs_pair[b][(slice(None), e) + rslice],
                            start=(i == 0), stop=(i == n - 1),
                        )
                        i += 1

            # ---- projections
            # q/k in [h(part), hc, l(free)] layout; v in [kl(part), klc, h] layout.
            if qk_exact:
                kT_h = big.tile([P, 2, KL], BF16, name="kT_h")
                kT_l = big.tile([P, 2, KL], BF16, name="kT_l")
                qT_h = big.tile([P, 2, L], BF16, name="qT_h")
                qT_l = big.tile([P, 2, L], BF16, name="qT_l")
            elif qk_fp16:
                kT_r = big.tile([P, 2, KL], F16, name="kT_r")
                qT_r = big.tile([P, 2, L], F16, name="qT_r")
            else:
                kT_r = big.tile([P, 2, KL], F32R, name="kT_r")
                qT_r = big.tile([P, 2, L], F32R, name="qT_r")
            if pv_exact:
                v_h = big.tile([P, NKC, HV], BF16, name="v_h")
                v_l = big.tile([P, NKC, HV], BF16, name="v_l")
            else:
                v_r = big.tile([P, NKC, HV], F32R, name="v_r")

            def split_store(ps, hi_ap, lo_ap):
                nc.scalar.copy(out=hi_ap, in_=ps)
                nc.vector.tensor_sub(lo_ap, ps, hi_ap)

            def emit_kq(hc, lt, which):
                ps = ps_st.tile([P, QT], F32, name="st", tag="st")
                w, e, dsts = (
                    (wk_t, embTk_t, (kT_h, kT_l) if qk_exact else (kT_r,))
                    if which == "k"
                    else (wq_t, embT_t, (qT_h, qT_l) if qk_exact else (qT_r,))
                )
                mm3(ps, w, e, (slice(hc * P, (hc + 1) * P),),
                    (slice(lt * QT, (lt + 1) * QT),))
                dst = (slice(None), hc, slice(lt * QT, (lt + 1) * QT))
                if qk_exact:
                    split_store(ps, dsts[0][dst], dsts[1][dst])
                else:
                    nc.scalar.copy(out=dsts[0][dst], in_=ps)

            def emit_v(kc):
                ps = ps_st.tile([P, QT], F32, name="st", tag="st")
                mm3(ps[:, :HV], embTk_t, wv_t, (slice(kc * P, (kc + 1) * P),),
                    (slice(None),))
                dst = (slice(None), kc, slice(None))
                if pv_exact:
                    split_store(ps[:, :HV], v_h[dst], v_l[dst])
                else:
                    nc.scalar.copy(out=v_r[dst], in_=ps[:, :HV])

            # Interleave the k/q tiles (PSUM->SBUF copy has slack) with the
            # v tiles (copy-bound) so the scalar/vector copies never gate PE.
            kq_tiles = [("k", hc, lt) for hc in range(2) for lt in range(NKT)]
            kq_tiles += [("q", hc, lt) for hc in range(2) for lt in range(NQTT)]
            vi = 0
            for i, (which, hc, lt) in enumerate(kq_tiles):
                emit_kq(hc, lt, which)
                want_v = ((i + 1) * NKC) // len(kq_tiles)
                while vi < want_v:
                    emit_v(vi)
                    vi += 1
            while vi < NKC:
                emit_v(vi)
                vi += 1

            # ---- attention
            # Software-pipelined emission: chunk kc's P@V matmuls are emitted
            # AFTER chunk kc+1's QK matmuls, so the PE always has independent
            # work in program order while the DVE mask-add + ACT exp of the
            # current chunk are still in flight.
            for qt in range(NQTT):
                pvs = [
                    ps_pv.tile([P, HV], F32, name=f"pv{j}", tag=f"pv{j}")
                    for j in range(4)
                ]
                qsl = slice(qt * QT, (qt + 1) * QT)
                pending_pv = None  # (kc, p-tiles) awaiting PV emission

                def emit_pv(kc, ptile):
                    for j in range(4):
                        jsl = slice(j * P, (j + 1) * P)
                        if pv_exact:
                            for t, (a, b) in enumerate(SPLIT3):
                                nc.tensor.matmul(
                                    pvs[j],
                                    lhsT=ptile[a][:, jsl],
                                    rhs=(v_h, v_l)[b][:, kc, :],
                                    start=(kc == 0 and t == 0),
                                    stop=(kc == NKC - 1 and t == 2),
                                )
                        else:
                            nc.tensor.matmul(
                                pvs[j],
                                lhsT=ptile[:, jsl],
                                rhs=v_r[:, kc, :],
                                start=(kc == 0), stop=(kc == NKC - 1),
                            )

                for kc in range(NKC):
                    ksl = slice(kc * P, (kc + 1) * P)
                    st = ps_st.tile([P, QT], F32, name="st", tag="st")
                    if qk_exact:
                        kp, qp = (kT_h, kT_l), (qT_h, qT_l)
                        n = 2 * len(SPLIT3)
                        i = 0
                        for a, b in SPLIT3:
                            for hc in range(2):
                                nc.tensor.matmul(
                                    st,
                                    lhsT=kp[a][:, hc, ksl],
                                    rhs=qp[b][:, hc, qsl],
                                    start=(i == 0), stop=(i == n - 1),
                                )
                                i += 1
                    else:
                        for hc in range(2):
                            nc.tensor.matmul(
                                st,
                                lhsT=kT_r[:, hc, ksl],
                                rhs=qT_r[:, hc, qsl],
                                start=(hc == 0), stop=(hc == 1),
                            )
                    if pending_pv is not None:
                        emit_pv(*pending_pv)
                    mt = mtp.tile([P, QT], BF16, name="mt", tag="mt")
                    nc.sync.dma_start(out=mt, in_=maskT[ksl, qsl])
                    nc.vector.tensor_tensor(out=st, in0=st, in1=mt, op=mybir.AluOpType.add)
                    if pv_exact:
                        pe = ptp.tile([P, QT], F32, name="pe", tag="pe")
                        nc.scalar.activation(out=pe, in_=st, func=mybir.ActivationFunctionType.Exp)
                        p_h = ptp.tile([P, QT], BF16, name="p_h", tag="p_h")
                        p_l = ptp.tile([P, QT], BF16, name="p_l", tag="p_l")
                        nc.vector.tensor_copy(p_h, pe)
                        nc.gpsimd.tensor_sub(p_l, pe, p_h)
                        pending_pv = (kc, (p_h, p_l))
                    else:
                        pt = ptp.tile([P, QT], F32R, name="pt", tag="pt")
                        nc.scalar.activation(out=pt, in_=st, func=mybir.ActivationFunctionType.Exp)
                        pending_pv = (kc, pt)
                emit_pv(*pending_pv)
                for j in range(4):
                    # Ship the unnormalized partial [sum p*v | sum p]; the
                    # host divides after combining the two key-halves.
                    ot = fin.tile([P, HID + 1], F32, name="ot", tag="ot")
                    nc.vector.tensor_copy(ot, pvs[j][:, :HID + 1])
                    row0 = (qt * 4 + j) * P
                    nc.sync.dma_start(out=out[row0:row0 + P, :], in_=ot)
    nc.finalize()
    return nc


def _get_nc():
    precision = os.environ.get("BASS_KERNEL_PRECISION", "balanced")
    key = f"nc_{precision}"
    if key not in _CACHE:
        _CACHE[key] = _build(precision)
    return _CACHE[key]


def _split_pair(x):
    hi = x.astype(BF)
    lo = (x - hi.astype(np.float32)).astype(BF)
    return hi, lo


def kernel(embedding, mask, Wq, bq, Wk, bk, Wv, bv):
    embedding = np.asarray(embedding, dtype=np.float32)
    mask = np.asarray(mask, dtype=np.float32)
    Wq = np.asarray(Wq, dtype=np.float32)
    Wk = np.asarray(Wk, dtype=np.float32)
    Wv = np.asarray(Wv, dtype=np.float32)
    bq = np.asarray(bq, dtype=np.float32)
    bk = np.asarray(bk, dtype=np.float32)
    bv = np.asarray(bv, dtype=np.float32)

    def pad_w(w, b, extra_one=False):
        wp = np.zeros((EPAD, HV if extra_one else HID), dtype=np.float32)
        wp[:EMB, :HID] = w
        wp[EMB, :HID] = b
        if extra_one:
            wp[EMB, HID] = 1.0
        return wp

    wq_h, wq_l = _split_pair(pad_w(Wq, bq))
    wk_h, wk_l = _split_pair(pad_w(Wk, bk))
    wv_h, wv_l = _split_pair(pad_w(Wv, bv, extra_one=True))

    in_maps = []
    for c in range(NCORES):
        b, half = divmod(c, 2)
        embT = np.zeros((EPAD, L), dtype=np.float32)
        embT[:EMB] = embedding[b].T
        embT[EMB] = 1.0
        e_h, e_l = _split_pair(embT)
        ksl = slice(half * KL, (half + 1) * KL)
        ek_h = np.ascontiguousarray(e_h[:, ksl])
        ek_l = np.ascontiguousarray(e_l[:, ksl])
        mT = np.ascontiguousarray(mask[b].T[ksl, :])
        mT = (mT * MASK_SCALE).astype(BF)
        in_maps.append({
            "embT_hi": e_h, "embT_lo": e_l,
            "embTk_hi": ek_h, "embTk_lo": ek_l,
            "wq_hi": wq_h, "wq_lo": wq_l,
            "wk_hi": wk_h, "wk_lo": wk_l,
            "wv_hi": wv_h, "wv_lo": wv_l,
            "maskT": mT,
        })

    nc = _get_nc()
    trace = bool(int(os.environ.get("BASS_KERNEL_TRACE", "0")))
    res = run_bass_kernel_spmd(nc, in_maps, core_ids=list(range(NCORES)), trace=trace)
    _CACHE["last_results"] = res

    full = np.empty((B, L, HID), dtype=np.float32)
    for b in range(B):
        r0 = res.results[2 * b]["out"].astype(np.float64)
        r1 = res.results[2 * b + 1]["out"].astype(np.float64)
        num = r0[:, :HID] + r1[:, :HID]
        den = r0[:, HID:] + r1[:, HID:]
        full[b] = (num / den).astype(np.float32)
    return full



# revision 19
# speedup vs baseline: 1.3670x; 1.3670x over previous
"""Single-head attention (B=4, L=4096, EMB=312, HID=256) on 8 NeuronCores.

Sharding: data-parallel over batch (4) x key-parallel (2) = 8 cores. Each
core handles ALL 4096 queries against its half of the keys and returns the
UNNORMALIZED partial [sum_k p*v | sum_k p] rows; the host combines the two
halves as (o1+o2)/(s1+s2).

To keep a single SPMD program, the host permutes each core's query columns
so the core's key half is always columns 0..2047 of its embT (the mask
columns and output rows are permuted identically; the host inverts the
permutation when combining).

Per-core device algorithm (everything single-pass):
  - embT/weights are fp32 in DRAM, tagged float32r: fp32r matmuls with
    free-dim >= 256 run at the bf16 PE rate, so projections, QK and the
    identity-free score pipeline all run 1 cycle/row.
  - embT carries a ones-row at index EMB and W* carry the bias in that row,
    so projections fold the bias in. Wv has 2 extra columns: ones (gives the
    softmax row-sum through the P@V matmul) and zero padding (even N).
  - Scores are computed transposed: sT[kl, ql] = kT-chunk^T @ qT. exp() runs
    directly on the score PSUM (no mask pre-add) writing bf16; the binary
    {0,1} bf16 mask is applied MULTIPLICATIVELY post-exp on the DVE, which
    hits the 2x packed-16-bit perf mode. The masked bf16 probabilities are
    the stationary operand of the bf16 P@V matmul.
  - PSUM->SBUF copies ride the otherwise-idle GPSIMD engine. The mask is
    one DMA per query tile ([2048 x 512] bf16) and the output one DMA per
    query tile, keeping the serial HWDGE ring (~630ns/descriptor-gen) cold.
  - P@V for chunk kc is emitted LOOK chunks behind its QK so the PE always
    has independent work while ACT exp + DVE mask-mult are in flight.
"""
import os

import numpy as np
import ml_dtypes

import concourse.bacc as bacc
import concourse.tile as tile
from concourse import mybir, bass2jax
from concourse.bass_utils import run_bass_kernel_spmd

# Debug aid (opt-in): surface real compile errors from the PJRT compile
# hook, which the C++ bridge otherwise swallows.
if os.environ.get("BASS_KERNEL_DEBUG"):
    import functools as _ft
    import traceback as _tb
    _orig_hook = bass2jax.neuronx_cc_hook
    @_ft.wraps(_orig_hook)
    def _dbg_hook(*args, **kwargs):
        try:
            return _orig_hook(*args, **kwargs)
        except BaseException:
            _tb.print_exc()
            raise
    bass2jax.neuronx_cc_hook = _dbg_hook

EMB, HID, B, L = 312, 256, 4, 4096
NCORES = 8
P = 128
KL = L // 2            # key rows per core (key-parallel halves)
EPAD = 384             # emb dim padded to 3 partition chunks; row EMB is the ones-row
HV = HID + 2           # v columns: HID values | ones | zero pad (even N for matmul)
QT = 512               # ql tile width (PSUM bank = 512 fp32)
NKC = KL // P          # 16 kl chunks per core
NQTT = L // QT         # 8 ql tiles per core (all queries)
NKT = KL // QT         # 4 l tiles for the k projection
LOOK = 3               # P@V emission lag (chunks) behind QK

F32 = mybir.dt.float32
F32R = mybir.dt.float32r
BF16 = mybir.dt.bfloat16
BF = ml_dtypes.bfloat16

_CACHE = {}


def _build():
    nc = bacc.Bacc(None)

    embT_d = nc.dram_tensor("embT", [EPAD, L], F32R, kind="ExternalInput")
    wq_d = nc.dram_tensor("wq", [EPAD, HID], F32R, kind="ExternalInput")
    wk_d = nc.dram_tensor("wk", [EPAD, HID], F32R, kind="ExternalInput")
    wv_d = nc.dram_tensor("wv", [EPAD, HV], F32R, kind="ExternalInput")
    maskT_d = nc.dram_tensor("maskT", [KL, L], BF16, kind="ExternalInput")
    out_d = nc.dram_tensor("out", [L, HID + 1], F32, kind="ExternalOutput")

    with tile.TileContext(nc) as tc:
        with (
            tc.tile_pool(name="big", bufs=1) as big,
            tc.tile_pool(name="mtp", bufs=2) as mtp,
            tc.tile_pool(name="pep", bufs=3) as pep,
            tc.tile_pool(name="ptp", bufs=5) as ptp,
            tc.tile_pool(name="fin", bufs=2) as fin,
            tc.tile_pool(name="ps_st", bufs=4, space="PSUM") as ps_st,
            tc.tile_pool(name="ps_pv", bufs=1, space="PSUM") as ps_pv,
        ):
            def cpn(d):
                return d.rearrange("(c p) n -> p c n", p=P)

            # ---- startup DMAs, interleaved so the first k-projection
            # (needs wk + embT block 0) can start as early as possible
            embT_t = big.tile([P, 3, L], F32R, name="embT_t")

            def load_block(b):
                sl = slice(b * QT, (b + 1) * QT)
                nc.sync.dma_start(out=embT_t[:, :, sl], in_=cpn(embT_d[:, sl]))

            wk_t = big.tile([P, 3, HID], F32R, name="wk_t")
            nc.sync.dma_start(out=wk_t[:, :, :P], in_=cpn(wk_d[:, :P]))
            # first k-projection needs only wk cols 0..127 + embT cols 0..255,
            # so those two small DMAs lead the ring
            sl0 = slice(0, QT // 2)
            nc.sync.dma_start(out=embT_t[:, :, sl0], in_=cpn(embT_d[:, sl0]))
            nc.sync.dma_start(out=wk_t[:, :, P:], in_=cpn(wk_d[:, P:]))
            sl1 = slice(QT // 2, QT)
            nc.sync.dma_start(out=embT_t[:, :, sl1], in_=cpn(embT_d[:, sl1]))
            wv_t = big.tile([P, 3, HV], F32R, name="wv_t")
            nc.sync.dma_start(out=wv_t, in_=cpn(wv_d[:, :]))
            load_block(1)
            wq_t = big.tile([P, 3, HID], F32R, name="wq_t")
            nc.sync.dma_start(out=wq_t, in_=cpn(wq_d[:, :]))
            load_block(2)

            # ---- projection destinations
            kT_t = big.tile([P, 2, KL], F32R, name="kT_t")
            qT_t = big.tile([P, 2, L], F32R, name="qT_t")
            v_t = big.tile([P, NKC, HV], BF16, name="v_t")

            def emit_kq(which, hc, c0, cw=QT):
                ps = ps_st.tile([P, QT], F32, name="st", tag="st")
                w, dstT = (wk_t, kT_t) if which == "k" else (wq_t, qT_t)
                lsl = slice(c0, c0 + cw)
                for e in range(3):
                    nc.tensor.matmul(
                        ps[:, :cw],
                        lhsT=w[:, e, hc * P:(hc + 1) * P],
                        rhs=embT_t[:, e, lsl],
                        start=(e == 0), stop=(e == 2),
                    )
                # q copies ride ACT (idle until the first exp); k/v ride DVE.
                # Splitting the two copy streams roughly halves the serial
                # projection-copy latency in front of the first attention QK.
                if which == "q":
                    nc.scalar.copy(dstT[:, hc, lsl], ps[:, :cw])
                else:
                    nc.vector.tensor_copy(dstT[:, hc, lsl], ps[:, :cw])

            def emit_v(kc):
                ps = ps_pv.tile([P, HV], F32, name="vps", tag=f"pv{kc % 4}")
                for e in range(3):
                    nc.tensor.matmul(
                        ps,
                        lhsT=embT_t[:, e, kc * P:(kc + 1) * P],
                        rhs=wv_t[:, e, :],
                        start=(e == 0), stop=(e == 2),
                    )
                nc.vector.tensor_copy(v_t[:, kc, :], ps)

            # k/v projections over the key half, block by block (first block
            # in 256-col halves to start compute on the leading quarter-DMA)
            emit_kq("k", 0, 0, QT // 2)
            emit_kq("k", 1, 0, QT // 2)
            emit_kq("k", 0, QT // 2, QT // 2)
            emit_kq("k", 1, QT // 2, QT // 2)
            for kc in range(4):
                emit_v(kc)
            for lt in range(1, NKT):
                emit_kq("k", 0, lt * QT)
                emit_kq("k", 1, lt * QT)
                for kc in range(4 * lt, 4 * lt + 4):
                    emit_v(kc)
            # q projections for the first half of the query tiles
            for lt in range(NKT):
                emit_kq("q", 0, lt * QT)
                emit_kq("q", 1, lt * QT)

            # ---- attention
            mask_tiles = {}

            def prefetch_mask(qt, part=None):
                if part is None or part == 0:
                    t = mtp.tile([P, NKC, QT], BF16, name="mt", tag="mt")
                    mask_tiles[qt] = t
                t = mask_tiles[qt]
                sl = slice(qt * QT, (qt + 1) * QT)
                if part is None:
                    nc.sync.dma_start(out=t, in_=cpn(maskT_d[:, sl]))
                else:
                    h = NKC // 2
                    csl = slice(part * h, (part + 1) * h)
                    nc.sync.dma_start(
                        out=t[:, csl, :],
                        in_=cpn(maskT_d[part * h * P:(part + 1) * h * P, sl]))

            # first mask half lands before embT block 3 so qt0's mask-mults
            # aren't gated on the whole 2MB mask transfer
            prefetch_mask(0, part=0)
            load_block(3)
            prefetch_mask(0, part=1)

            # One flat software pipeline over all (qt, kc) chunks: QK/exp/mult
            # of chunk t are emitted together; P@V of chunk t-LOOK follows, so
            # qt boundaries interleave naturally and the PE never drains.
            sts, pts, pvs = {}, {}, {}

            def qk_expmul(t):
                qt, kc = divmod(t, NKC)
                st = ps_st.tile([P, QT], F32, name="st", tag="st")
                ksl = slice(kc * P, (kc + 1) * P)
                qsl = slice(qt * QT, (qt + 1) * QT)
                for hc in range(2):
                    nc.tensor.matmul(
                        st,
                        lhsT=kT_t[:, hc, ksl],
                        rhs=qT_t[:, hc, qsl],
                        start=(hc == 0), stop=(hc == 1),
                    )
                pe = pep.tile([P, QT], BF16, name="pe", tag="pe")
                nc.scalar.activation(
                    out=pe, in_=st, func=mybir.ActivationFunctionType.Exp,
                )
                pt = ptp.tile([P, QT], BF16, name="pt", tag="pt")
                nc.vector.tensor_tensor(
                    out=pt, in0=pe, in1=mask_tiles[qt][:, kc, :],
                    op=mybir.AluOpType.mult,
                )
                pts[t] = pt

            def pv(t):
                qt, kc = divmod(t, NKC)
                if kc == 0:
                    pvs[qt] = [
                        ps_pv.tile([P, HV], F32, name="pv", tag=f"pv{j}")
                        for j in range(4)
                    ]
                pt = pts.pop(t)
                last = kc == NKC - 1
                ft = None
                for j in range(4):
                    nc.tensor.matmul(
                        pvs[qt][j],
                        lhsT=pt[:, j * P:(j + 1) * P],
                        rhs=v_t[:, kc, :],
                        start=(kc == 0), stop=last,
                    )
                    if last:
                        # finish column group j as soon as its accumulation
                        # ends: copies alternate ACT/DVE, DMA per pair
                        if j % 2 == 0:
                            ft = fin.tile([P, 2, HID + 1], F32, name="ft",
                                          tag=f"ft{j // 2}")
                            nc.scalar.copy(ft[:, 0, :], pvs[qt][j][:, :HID + 1])
                        else:
                            nc.vector.tensor_copy(ft[:, 1, :],
                                                  pvs[qt][j][:, :HID + 1])
                            r0 = qt * QT + (j - 1) * P
                            nc.sync.dma_start(
                                out=cpn(out_d[r0:r0 + 2 * P, :]), in_=ft)
                if last:
                    del pvs[qt], mask_tiles[qt]

            T = NQTT * NKC
            for t in range(T + LOOK):
                if t < T:
                    qt, kc = divmod(t, NKC)
                    if kc == 0:
                        # overlap second-half embT blocks and the next mask
                        # prefetch with this qt's compute (block DMA first so
                        # it isn't queued behind the 2MB mask transfer)
                        if qt < NKT:
                            load_block(NKT + qt)
                        if qt < NQTT - 1:
                            prefetch_mask(qt + 1)
                    elif kc == 6 and qt < NKT:
                        # late q projections: by now embT block 4+qt has
                        # landed, so these never block the PE FIFO
                        emit_kq("q", 0, (NKT + qt) * QT)
                        emit_kq("q", 1, (NKT + qt) * QT)
                    qk_expmul(t)
                if t >= LOOK:
                    pv(t - LOOK)
    nc.finalize()
    return nc


def _get_nc():
    if "nc" not in _CACHE:
        _CACHE["nc"] = _build()
    return _CACHE["nc"]


def kernel(embedding, mask, Wq, bq, Wk, bk, Wv, bv):
    embedding = np.asarray(embedding, dtype=np.float32)
    mask = np.asarray(mask, dtype=np.float32)
    Wq = np.asarray(Wq, dtype=np.float32)
    Wk = np.asarray(Wk, dtype=np.float32)
    Wv = np.asarray(Wv, dtype=np.float32)
    bq = np.asarray(bq, dtype=np.float32)
    bk = np.asarray(bk, dtype=np.float32)
    bv = np.asarray(bv, dtype=np.float32)

    def pad_w(w, b, extra_one=False):
        wp = np.zeros((EPAD, HV if extra_one else HID), dtype=np.float32)
        wp[:EMB, :HID] = w
        wp[EMB, :HID] = b
        if extra_one:
            wp[EMB, HID] = 1.0
        return wp

    wq_p = pad_w(Wq, bq)
    wk_p = pad_w(Wk, bk)
    wv_p = pad_w(Wv, bv, extra_one=True)

    perms = [np.arange(L), np.concatenate([np.arange(KL, L), np.arange(KL)])]

    in_maps = []
    for c in range(NCORES):
        b, half = divmod(c, 2)
        perm = perms[half]
        embT = np.zeros((EPAD, L), dtype=np.float32)
        embT[:EMB] = embedding[b].T[:, perm]
        embT[EMB] = 1.0
        ksl = slice(half * KL, (half + 1) * KL)
        mT = np.ascontiguousarray(mask[b].T[ksl][:, perm])
        # binary {1=masked} -> multiplicative {0=masked, 1=keep}
        mT = (1.0 - mT).astype(BF)
        in_maps.append({
            "embT": embT,
            "wq": wq_p, "wk": wk_p, "wv": wv_p,
            "maskT": mT,
        })

    nc = _get_nc()
    trace = bool(int(os.environ.get("BASS_KERNEL_TRACE", "0")))
    res = run_bass_kernel_spmd(nc, in_maps, core_ids=list(range(NCORES)), trace=trace)
    _CACHE["last_results"] = res

    full = np.empty((B, L, HID), dtype=np.float32)
    for b in range(B):
        r0 = res.results[2 * b]["out"].astype(np.float64)
        r1 = res.results[2 * b + 1]["out"].astype(np.float64)[perms[1]]
        num = r0[:, :HID] + r1[:, :HID]
        den = r0[:, HID:] + r1[:, HID:]
        full[b] = (num / den).astype(np.float32)
    return full


# revision 35
# speedup vs baseline: 1.4159x; 1.0358x over previous
"""Single-head attention (B=4, L=4096, EMB=312, HID=256) on 8 NeuronCores.

Sharding: data-parallel over batch (4) x key-parallel (2) = 8 cores. Each
core handles ALL 4096 queries against its half of the keys and returns the
UNNORMALIZED partial [sum_k p*v | sum_k p] rows; the host combines the two
halves as (o1+o2)/(s1+s2).

To keep a single SPMD program, the host permutes each core's query columns
so the core's key half is always columns 0..2047 of its embT (the mask
columns and output rows are permuted identically; the host inverts the
permutation when combining).

Per-core device algorithm (everything single-pass):
  - embT/weights are fp32 in DRAM, tagged float32r: fp32r matmuls with
    free-dim >= 256 run at the bf16 PE rate, so projections, QK and the
    identity-free score pipeline all run 1 cycle/row.
  - embT carries a ones-row at index EMB and W* carry the bias in that row,
    so projections fold the bias in. Wv has 2 extra columns: ones (gives the
    softmax row-sum through the P@V matmul) and zero padding (even N).
  - Scores are computed transposed: sT[kl, ql] = kT-chunk^T @ qT. exp() runs
    directly on the score PSUM (no mask pre-add) writing bf16; the binary
    {0,1} bf16 mask is applied MULTIPLICATIVELY post-exp on the DVE, which
    hits the 2x packed-16-bit perf mode. The masked bf16 probabilities are
    the stationary operand of the bf16 P@V matmul.
  - PSUM->SBUF copies ride the otherwise-idle GPSIMD engine. The mask is
    one DMA per query tile ([2048 x 512] bf16) and the output one DMA per
    query tile, keeping the serial HWDGE ring (~630ns/descriptor-gen) cold.
  - P@V for chunk kc is emitted LOOK chunks behind its QK so the PE always
    has independent work while ACT exp + DVE mask-mult are in flight.
"""
import os

import numpy as np
import ml_dtypes

import concourse.bacc as bacc
import concourse.tile as tile
from concourse import mybir, bass2jax
from concourse.bass_utils import run_bass_kernel_spmd

# Debug aid (opt-in): surface real compile errors from the PJRT compile
# hook, which the C++ bridge otherwise swallows.
if os.environ.get("BASS_KERNEL_DEBUG"):
    import functools as _ft
    import traceback as _tb
    _orig_hook = bass2jax.neuronx_cc_hook
    @_ft.wraps(_orig_hook)
    def _dbg_hook(*args, **kwargs):
        try:
            return _orig_hook(*args, **kwargs)
        except BaseException:
            _tb.print_exc()
            raise
    bass2jax.neuronx_cc_hook = _dbg_hook

EMB, HID, B, L = 312, 256, 4, 4096
NCORES = 8
P = 128
KL = L // 2            # key rows per core (key-parallel halves)
EPAD = 384             # emb dim padded to 3 partition chunks; row EMB is the ones-row
HV = HID + 2           # v columns: HID values | ones | zero pad (even N for matmul)
QT = 512               # ql tile width (PSUM bank = 512 fp32)
NKC = KL // P          # 16 kl chunks per core
NQTT = L // QT         # 8 ql tiles per core (all queries)
NKT = KL // QT         # 4 l tiles for the k projection
LOOK = 3               # P@V emission lag (chunks) behind QK

F32 = mybir.dt.float32
F32R = mybir.dt.float32r
BF16 = mybir.dt.bfloat16
BF = ml_dtypes.bfloat16

_CACHE = {}


def _build():
    nc = bacc.Bacc(None)

    ER = EMB + 1
    E2 = ER - 2 * P
    embT_d = nc.dram_tensor("embT", [ER, L], F32R, kind="ExternalInput")
    wq_d = nc.dram_tensor("wq", [ER, HID], F32R, kind="ExternalInput")
    wk_d = nc.dram_tensor("wk", [ER, HID], F32R, kind="ExternalInput")
    wv_d = nc.dram_tensor("wv", [ER, HV], F32R, kind="ExternalInput")
    maskT_d = nc.dram_tensor("maskT", [KL, L], BF16, kind="ExternalInput")
    out_d = nc.dram_tensor("out", [L, HID + 1], F32, kind="ExternalOutput")

    with tile.TileContext(nc) as tc:
        with (
            tc.tile_pool(name="big", bufs=1) as big,
            tc.tile_pool(name="mtp", bufs=2) as mtp,
            tc.tile_pool(name="pep", bufs=3) as pep,
            tc.tile_pool(name="ptp", bufs=5) as ptp,
            tc.tile_pool(name="fin", bufs=2) as fin,
            tc.tile_pool(name="ps_st", bufs=4, space="PSUM") as ps_st,
            tc.tile_pool(name="ps_pv", bufs=1, space="PSUM") as ps_pv,
        ):
            def cpn(d):
                return d.rearrange("(c p) n -> p c n", p=P)

            # ---- PE warmup during the startup DMA wait
            warm = big.tile([P, QT], F32R, name="warm")
            nc.gpsimd.memset(warm.bitcast(F32), 0.0)
            wps = ps_st.tile([P, QT], F32, name="st", tag="st")
            for _ in range(10):
                nc.tensor.matmul(wps, lhsT=warm[:, :P], rhs=warm,
                                 start=True, stop=True)

            # ---- startup DMAs (trimmed rows, per-chunk writes)
            embT_t = big.tile([P, 3, L], F32R, name="embT_t")
            wk_t = big.tile([P, 3, HID], F32R, name="wk_t")
            wv_t = big.tile([P, 3, HV], F32R, name="wv_t")
            wq_t = big.tile([P, 3, HID], F32R, name="wq_t")
            for wt in (wk_t, wv_t, wq_t):
                nc.gpsimd.memset(wt[:, 2, :].bitcast(F32), 0.0)
            nc.gpsimd.memset(embT_t[:, 2, :].bitcast(F32), 0.0)

            def load_trim(dst3, dram, csl):
                nc.sync.dma_start(
                    out=dst3[:, 0:2, :],
                    in_=dram[0:2 * P, csl].rearrange("(c p) n -> p c n", p=P))
                nc.sync.dma_start(out=dst3[0:E2, 2, :], in_=dram[2 * P:ER, csl])

            def load_block(b):
                sl = slice(b * QT, (b + 1) * QT)
                load_trim(embT_t[:, :, sl], embT_d, sl)

            load_trim(wk_t[:, :, :P], wk_d, slice(0, P))
            load_trim(embT_t[:, :, 0:QT // 2], embT_d, slice(0, QT // 2))
            load_trim(wk_t[:, :, P:], wk_d, slice(P, HID))
            load_trim(embT_t[:, :, QT // 2:QT], embT_d, slice(QT // 2, QT))
            load_trim(wv_t, wv_d, slice(0, HV))
            load_block(1)
            load_trim(wq_t, wq_d, slice(0, HID))
            load_block(2)
            load_block(3)

            # ---- projection destinations
            kT_t = big.tile([P, 2, KL], F32R, name="kT_t")
            qT_t = big.tile([P, 2, L], F32R, name="qT_t")
            v_t = big.tile([P, NKC, HV], BF16, name="v_t")

            def emit_kq(which, hc, c0, cw=QT):
                ps = ps_st.tile([P, QT], F32, name="st", tag="st")
                w, dstT = (wk_t, kT_t) if which == "k" else (wq_t, qT_t)
                lsl = slice(c0, c0 + cw)
                for e in range(3):
                    nc.tensor.matmul(
                        ps[:, :cw],
                        lhsT=w[:, e, hc * P:(hc + 1) * P],
                        rhs=embT_t[:, e, lsl],
                        start=(e == 0), stop=(e == 2),
                    )
                if which == "q":
                    nc.scalar.copy(dstT[:, hc, lsl], ps[:, :cw])
                else:
                    nc.vector.tensor_copy(dstT[:, hc, lsl], ps[:, :cw])

            def emit_v(kc):
                ps = ps_pv.tile([P, HV], F32, name="vps", tag=f"pv{kc % 4}")
                for e in range(3):
                    nc.tensor.matmul(
                        ps,
                        lhsT=embT_t[:, e, kc * P:(kc + 1) * P],
                        rhs=wv_t[:, e, :],
                        start=(e == 0), stop=(e == 2),
                    )
                nc.vector.tensor_copy(v_t[:, kc, :], ps)

            def warm_fill(n):
                for _ in range(n):
                    nc.tensor.matmul(wps, lhsT=warm[:, :P], rhs=warm,
                                     start=True, stop=True)

            emit_kq("k", 0, 0, QT // 2)
            emit_kq("k", 1, 0, QT // 2)
            emit_kq("k", 0, QT // 2, QT // 2)
            emit_kq("k", 1, QT // 2, QT // 2)
            warm_fill(3)
            for kc in range(4):
                emit_v(kc)
            warm_fill(3)
            for lt in range(1, NKT):
                emit_kq("k", 0, lt * QT)
                emit_kq("k", 1, lt * QT)
                if lt == 1:
                    warm_fill(2)
                for kc in range(4 * lt, 4 * lt + 4):
                    emit_v(kc)
            # q projections for the first half of the query tiles
            for lt in range(NKT):
                emit_kq("q", 0, lt * QT)
                emit_kq("q", 1, lt * QT)

            # ---- attention
            mask_tiles = {}
            HM = NKC // 2

            def prefetch_mask(qt, part=None):
                if part is None:
                    prefetch_mask(qt, 0)
                    prefetch_mask(qt, 1)
                    return
                t = mtp.tile([P, HM, QT], BF16, name="mt", tag=f"mt{part}")
                mask_tiles[qt, part] = t
                sl = slice(qt * QT, (qt + 1) * QT)
                nc.sync.dma_start(
                    out=t,
                    in_=cpn(maskT_d[part * HM * P:(part + 1) * HM * P, sl]))

            prefetch_mask(0)

            # One flat software pipeline over all (qt, kc) chunks: QK/exp/mult
            # of chunk t are emitted together; P@V of chunk t-LOOK follows, so
            # qt boundaries interleave naturally and the PE never drains.
            sts, pts, pvs = {}, {}, {}

            def qk_expmul(t):
                qt, kc = divmod(t, NKC)
                st = ps_st.tile([P, QT], F32, name="st", tag="st")
                ksl = slice(kc * P, (kc + 1) * P)
                qsl = slice(qt * QT, (qt + 1) * QT)
                for hc in range(2):
                    nc.tensor.matmul(
                        st,
                        lhsT=kT_t[:, hc, ksl],
                        rhs=qT_t[:, hc, qsl],
                        start=(hc == 0), stop=(hc == 1),
                    )
                pe = pep.tile([P, QT], BF16, name="pe", tag="pe")
                nc.scalar.activation(
                    out=pe, in_=st, func=mybir.ActivationFunctionType.Exp,
                )
                pt = ptp.tile([P, QT], BF16, name="pt", tag="pt")
                nc.vector.tensor_tensor(
                    out=pt, in0=pe, in1=mask_tiles[qt, kc // HM][:, kc % HM, :],
                    op=mybir.AluOpType.mult,
                )
                pts[t] = pt

            def pv(t):
                qt, kc = divmod(t, NKC)
                if kc == 0:
                    pvs[qt] = [
                        ps_pv.tile([P, HV], F32, name="pv", tag=f"pv{j}")
                        for j in range(4)
                    ]
                pt = pts.pop(t)
                last = kc == NKC - 1
                ft = None
                for j in range(4):
                    nc.tensor.matmul(
                        pvs[qt][j],
                        lhsT=pt[:, j * P:(j + 1) * P],
                        rhs=v_t[:, kc, :],
                        start=(kc == 0), stop=last,
                    )
                    if last:
                        # finish column group j as soon as its accumulation
                        # ends: copies alternate ACT/DVE, DMA per pair
                        if j % 2 == 0:
                            ft = fin.tile([P, 2, HID + 1], F32, name="ft",
                                          tag=f"ft{j // 2}")
                            nc.scalar.copy(ft[:, 0, :], pvs[qt][j][:, :HID + 1])
                        else:
                            nc.vector.tensor_copy(ft[:, 1, :],
                                                  pvs[qt][j][:, :HID + 1])
                            r0 = qt * QT + (j - 1) * P
                            nc.sync.dma_start(
                                out=cpn(out_d[r0:r0 + 2 * P, :]), in_=ft)
                if last:
                    del pvs[qt], mask_tiles[qt, 0], mask_tiles[qt, 1]

            T = NQTT * NKC
            for t in range(T + LOOK):
                if t < T:
                    qt, kc = divmod(t, NKC)
                    if kc == 0:
                        if qt < NKT:
                            load_block(NKT + qt)
                        if qt < NQTT - 1:
                            prefetch_mask(qt + 1)
                    elif kc == 6 and qt < NKT:
                        emit_kq("q", 0, (NKT + qt) * QT)
                        emit_kq("q", 1, (NKT + qt) * QT)
                    qk_expmul(t)
                if t >= LOOK:
                    pv(t - LOOK)
    nc.finalize()
    return nc


def _get_nc():
    if "nc" not in _CACHE:
        _CACHE["nc"] = _build()
    return _CACHE["nc"]


def kernel(embedding, mask, Wq, bq, Wk, bk, Wv, bv):
    embedding = np.asarray(embedding, dtype=np.float32)
    mask = np.asarray(mask, dtype=np.float32)
    Wq = np.asarray(Wq, dtype=np.float32)
    Wk = np.asarray(Wk, dtype=np.float32)
    Wv = np.asarray(Wv, dtype=np.float32)
    bq = np.asarray(bq, dtype=np.float32)
    bk = np.asarray(bk, dtype=np.float32)
    bv = np.asarray(bv, dtype=np.float32)

    def pad_w(w, b, extra_one=False):
        wp = np.zeros((EMB + 1, HV if extra_one else HID), dtype=np.float32)
        wp[:EMB, :HID] = w
        wp[EMB, :HID] = b
        if extra_one:
            wp[EMB, HID] = 1.0
        return wp

    wq_p = pad_w(Wq, bq)
    wk_p = pad_w(Wk, bk)
    wv_p = pad_w(Wv, bv, extra_one=True)

    perms = [np.arange(L), np.concatenate([np.arange(KL, L), np.arange(KL)])]

    in_maps = []
    for c in range(NCORES):
        b, half = divmod(c, 2)
        perm = perms[half]
        embT = np.empty((EMB + 1, L), dtype=np.float32)
        embT[:EMB] = embedding[b].T[:, perm]
        embT[EMB] = 1.0
        ksl = slice(half * KL, (half + 1) * KL)
        mT = np.ascontiguousarray(mask[b].T[ksl][:, perm])
        # binary {1=masked} -> multiplicative {0=masked, 1=keep}
        mT = (1.0 - mT).astype(BF)
        in_maps.append({
            "embT": embT,
            "wq": wq_p, "wk": wk_p, "wv": wv_p,
            "maskT": mT,
        })

    nc = _get_nc()
    trace = bool(int(os.environ.get("BASS_KERNEL_TRACE", "0")))
    res = run_bass_kernel_spmd(nc, in_maps, core_ids=list(range(NCORES)), trace=trace)
    _CACHE["last_results"] = res

    full = np.empty((B, L, HID), dtype=np.float32)
    for b in range(B):
        r0 = res.results[2 * b]["out"].astype(np.float64)
        r1 = res.results[2 * b + 1]["out"].astype(np.float64)[perms[1]]
        num = r0[:, :HID] + r1[:, :HID]
        den = r0[:, HID:] + r1[:, HID:]
        full[b] = (num / den).astype(np.float32)
    return full
